# revision 30
# baseline (speedup 1.0000x reference)
"""Trainium2 Bass kernel for the Guided-Conv problem.

Math (per independent sample n, of NB = 4096):
  g_n, d_n : 24x24x9 patches of guidance / depth.
  c_n      = conv2d(g_n, conv_w, stride 8, SAME) + conv_b        -> 3x3x9
  k_n[i]   = c_n[:, :, i] / max(||c_n[:, :, i]||_2, 1)           (per-channel 3x3 filter)
  gap_n    = mean(g_n, (y, x))                                   -> 9
  W2_n     = (gap_n @ dense_w + dense_b).reshape(9, 9)           (i2 -> o2)
  r2_n[o]  = 1 / max(||W2_n[:, o]||_2, 1)
  out_n    = (depthwise(d_n, k_n) @ W2_n) * r2_n                 -> 24x24x9

Device strategy (per core: 512 samples + 6 pad = 37 groups of 14):
  Partition layout q = n_local*9 + ch on 126 partitions; free = pixels.
  - Kernel generation (c_n, W2_n) via block-diagonal matmuls: K = (n,ch),
    lhsT = kron(eye(14), w) built on host, so 14 samples per matmul.
  - Depthwise(3x3) + 1x1 fused: out[(n,o), pix] = sum_{t,i} BD_t[(n,i),(n,o)]
    * d_pad[(n,i), pix+t], 9 tap-matmuls accumulating in PSUM, float32r.
    BD_t = mask (.) (W2row-bcast) (.) k[:, t]  -- one DVE scalar_tensor_tensor.
  - r2 applied for free as the per-partition ACT scale on the PSUM->SBUF copy.
Host does all layout (patch extraction, channel de-interleave, zero-pad) --
this keeps every DMA contiguous in >=2KB runs.
"""

import numpy as np
import ml_dtypes

import concourse.bass as bass
from concourse import bacc
import concourse.mybir as mybir
from concourse.tile import TileContext
from concourse.bass_utils import run_bass_kernel_spmd

BF16 = ml_dtypes.bfloat16

F = 9          # channels
P = 24         # patch size
PADW = 26      # padded patch width (SAME conv, pad 1)
KS = 3         # generated kernel size
NCORES = 8
NL = 14        # samples per group
Q = NL * F     # 126 used partitions
NGROUP = 37    # groups per core (36 full + 1 padded)
SPC = NGROUP * NL  # 518 sample slots per core (512 real)
PIX = P * P        # 576
PPIX = PADW * PADW  # 676
HALF = PIX // 2    # 288, pixels per PSUM chunk (<=512 fp32/bank)
SUPER = [10, 9, 9, 9]  # weight-gen supertile sizes (sum = 37)

F32 = mybir.dt.float32
BF = mybir.dt.bfloat16


def build_program():
    nc = bacc.Bacc("TRN2", target_bir_lowering=False, debug=False,
                   num_devices=NCORES)

    CW = 3 * F * Q + Q  # lhsA | lhsD | lhsD2 | mask = 3528 bf16/partition
    gin = nc.dram_tensor("gin", [Q, NGROUP, PIX], BF, kind="ExternalInput").ap()
    din = nc.dram_tensor("din", [Q, NGROUP, PPIX], BF, kind="ExternalInput").ap()
    cpack = nc.dram_tensor("cpack", [Q + 1, CW], BF, kind="ExternalInput").ap()
    convbd = nc.dram_tensor("convb", [Q, 1], F32, kind="ExternalInput").ap()
    outd = nc.dram_tensor("out", [Q, NGROUP, PIX], BF, kind="ExternalOutput").ap()

    with TileContext(nc) as tc:
        with (
            nc.allow_low_precision(reason="bf16 pipeline; tol 2e-2"),
            tc.tile_pool(name="consts", bufs=1) as cpool,
            tc.tile_pool(name="gpool", bufs=2) as gpool,
            tc.tile_pool(name="dpool", bufs=8) as dpool,
            tc.tile_pool(name="opool", bufs=6) as opool,
            tc.tile_pool(name="scrap", bufs=2) as scpool,
            tc.tile_pool(name="small", bufs=1) as spool,
            tc.tile_pool(name="bd", bufs=12) as bdpool,
            tc.tile_pool(name="ps_c", bufs=1, space="PSUM") as pcpool,
            tc.tile_pool(name="ps_d", bufs=1, space="PSUM") as pdpool,
            tc.tile_pool(name="ps_main", bufs=3, space="PSUM") as pmpool,
        ):
            # ---- constants: ONE packed DMA (cheap issue, lands early) ----
            cp = cpool.tile([Q + 1, CW], BF, tag="cpack")
            nc.sync.dma_start(out=cp, in_=cpack)
            lhsA_sb = cp[0:Q, 0:F * Q].rearrange("p (t k) -> p t k", k=Q)
            lhsD_sb = cp[:, F * Q:2 * F * Q].rearrange("p (j k) -> p j k", k=Q)
            lhsD2_sb = cp[:, 2 * F * Q:3 * F * Q].rearrange("p (j k) -> p j k", k=Q)
            mask_sb = cp[0:Q, 3 * F * Q:3 * F * Q + Q].rearrange(
                "p (a b) -> p a b", b=F)
            convb_sb = cpool.tile([Q, 1], F32, tag="convb")
            nc.gpsimd.dma_start(out=convb_sb, in_=convbd)

            # ---- persistent per-core small tensors ----
            craw = spool.tile([Q, NGROUP, F], F32, tag="craw")     # c + conv_b
            knorm = spool.tile([Q, NGROUP, F], BF, tag="knorm")    # normalized taps
            w2 = spool.tile([Q, NGROUP, F], BF, tag="w2")          # raw W2 (D2 layout)
            r2 = spool.tile([Q, NGROUP], F32, tag="r2")            # 1/max(n2,1)
            r1 = spool.tile([Q, NGROUP], F32, tag="r1")            # 1/max(n1,1)
            sq = spool.tile([Q, NGROUP, F], F32, tag="sq")         # scratch squares
            s1 = spool.tile([Q, NGROUP], F32, tag="s1")            # scratch sums

            starts = []
            _g = 0
            for ng in SUPER:
                starts.append(_g)
                _g += ng

            pre = {}

            def emit_gin(si):
                # prefetch guidance for supertile si + its gap reduction
                ngi = SUPER[si]
                gsli = slice(starts[si], starts[si] + ngi)
                gsb = gpool.tile([Q, ngi * PIX], BF, tag="gsb")
                nc.sync.dma_start(out=gsb,
                                  in_=gin[:, gsli].rearrange("p g f -> p (g f)"))
                # gap: per-group pixel SUM (the 1/576 mean scale is folded
                # into lhsD/lhsD2 on the host). Row 126 must read 1.0 in the
                # K=127 dense matmuls, so memset the whole tile first.
                gap = spool.tile([128, ngi], BF, tag="gap", bufs=2)
                nc.vector.memset(gap, 1.0)
                nc.vector.tensor_reduce(
                    out=gap[0:Q, :],
                    in_=gsb.rearrange("p (g f) -> p g f", g=ngi),
                    axis=mybir.AxisListType.X, op=mybir.AluOpType.add)
                pre[si] = (gsb, gap)

            emit_gin(0)
            for si, ng in enumerate(SUPER):
                g0 = starts[si]
                gsl = slice(g0, g0 + ng)
                gsb, gap = pre.pop(si)

                # step A: strided conv -> c, 9 accumulated BD matmuls
                psc = pcpool.tile([Q, ng, F], F32, tag="psc")
                gwin = gsb.rearrange(
                    "p (g oy yr ox xr) -> p g oy ox yr xr",
                    g=ng, oy=KS, yr=8, ox=KS, xr=8)
                for t in range(KS * KS):
                    ky, kx = divmod(t, KS)
                    nc.tensor.matmul(
                        psc,
                        lhsT=lhsA_sb[:, t, :],
                        rhs=gwin[:, :, :, :, ky, kx],
                        start=(t == 0), stop=(t == KS * KS - 1),
                        skip_group_check=True)

                # craw = psc + conv_b (per-partition bias)
                nc.scalar.activation(
                    out=craw[:, gsl, :], in_=psc,
                    func=mybir.ActivationFunctionType.Identity,
                    bias=convb_sb, scale=1.0)

                # dense layer, both layouts (D for the norm, D2 for the values)
                psDall = pdpool.tile([Q, 2, F, ng], F32, tag="psDall")
                psD = psDall[:, 0]
                psD2 = psDall[:, 1]
                for j in range(F):
                    nc.tensor.matmul(psD[:, j, :], lhsT=lhsD_sb[:, j, :],
                                     rhs=gap[0:Q + 1, :],
                                     start=True, stop=True,
                                     skip_group_check=True)
                for j in range(F):
                    nc.tensor.matmul(psD2[:, j, :], lhsT=lhsD2_sb[:, j, :],
                                     rhs=gap[0:Q + 1, :],
                                     start=True, stop=True,
                                     skip_group_check=True)

                # r2 = 1/max(||W2[:,o]||, 1):  sum_i2 D^2 per (n,o2)
                nc.scalar.square(out=sq[:, gsl, :],
                                 in_=psD.rearrange("p i g -> p g i"))
                nc.vector.tensor_reduce(
                    out=s1[:, gsl], in_=sq[:, gsl, :],
                    axis=mybir.AxisListType.X, op=mybir.AluOpType.add)
                nc.scalar.sqrt(out=s1[:, gsl], in_=s1[:, gsl])
                nc.vector.tensor_scalar_max(r2[:, gsl], s1[:, gsl], 1.0)
                nc.vector.reciprocal(r2[:, gsl], r2[:, gsl])

                # W2 raw values, group-major
                nc.scalar.copy(out=w2[:, gsl, :],
                               in_=psD2.rearrange("p o g -> p g o"))

                # r1 = 1/max(||c||, 1) per (n, ch); knorm = craw * r1
                nc.scalar.square(out=sq[:, gsl, :], in_=craw[:, gsl, :])
                nc.vector.tensor_reduce(
                    out=s1[:, gsl], in_=sq[:, gsl, :],
                    axis=mybir.AxisListType.X, op=mybir.AluOpType.add)
                nc.scalar.sqrt(out=s1[:, gsl], in_=s1[:, gsl])
                nc.vector.tensor_scalar_max(r1[:, gsl], s1[:, gsl], 1.0)
                nc.vector.reciprocal(r1[:, gsl], r1[:, gsl])
                nc.vector.tensor_mul(
                    out=knorm[:, gsl, :], in0=craw[:, gsl, :],
                    in1=r1[:, gsl].unsqueeze(2).broadcast_to([Q, ng, F]))

                # ---------- main conv, groups in pairs (fewer DMA setups) ----
                for pi, p0 in enumerate(range(g0, g0 + ng, 2)):
                    npair = min(2, g0 + ng - p0)
                    if pi == 1 and si + 1 < len(SUPER):
                        emit_gin(si + 1)
                    dsb = dpool.tile([Q, 2 * PPIX], BF, tag="dsb")
                    nc.sync.dma_start(
                        out=dsb[:, :npair * PPIX],
                        in_=din[:, p0:p0 + npair].rearrange("p g f -> p (g f)"))
                    osb = opool.tile([Q, 2 * PIX], BF, tag="osb")

                    # W2 block-diagonal base for the pair: bdw = mask (.) w2
                    # (one DVE tensor_tensor, 2x mode), then per-tap scaling
                    # bd_t = bdw * k_t via tensor_scalar (4x) spread across
                    # DVE / ACT / GpSimd to balance engine load.
                    bdw = bdpool.tile([Q, 2, NL, F], BF, tag="bdw")
                    nc.vector.tensor_tensor(
                        out=bdw[:, :npair],
                        in0=mask_sb.unsqueeze(1).broadcast_to([Q, npair, NL, F]),
                        in1=w2[:, p0:p0 + npair, :].unsqueeze(2)
                            .broadcast_to([Q, npair, NL, F]),
                        op=mybir.AluOpType.mult)

                    for gl in range(npair):
                        g = p0 + gl
                        drows = dsb[:, gl * PPIX:(gl + 1) * PPIX].rearrange(
                            "p (r c) -> p r c", c=PADW)
                        # bd_t = bdw * k_t, batched: taps 0-5 in one DVE
                        # tensor_tensor, taps 6-8 on GpSimd.
                        bda = bdpool.tile([Q, 6, NL, F], BF, tag="bda")
                        nc.vector.tensor_tensor(
                            out=bda,
                            in0=bdw[:, gl].unsqueeze(1)
                                .broadcast_to([Q, 6, NL, F]),
                            in1=knorm[:, g, 0:6].unsqueeze(2).unsqueeze(3)
                                .broadcast_to([Q, 6, NL, F]),
                            op=mybir.AluOpType.mult)
                        bdb = bdpool.tile([Q, 3, NL, F], BF, tag="bdb")
                        nc.gpsimd.tensor_tensor(
                            out=bdb,
                            in0=bdw[:, gl].unsqueeze(1)
                                .broadcast_to([Q, 3, NL, F]),
                            in1=knorm[:, g, 6:9].unsqueeze(2).unsqueeze(3)
                                .broadcast_to([Q, 3, NL, F]),
                            op=mybir.AluOpType.mult)

                        pm = pmpool.tile([Q, 2, 512], F32, tag="pm")
                        for t in range(KS * KS):
                            ky, kx = divmod(t, KS)
                            src = bda[:, t] if t < 6 else bdb[:, t - 6]
                            lhsT = src.rearrange("p a b -> p (a b)")
                            for h in range(2):
                                rhs = drows[:, h * 12 + ky:h * 12 + ky + 12,
                                            kx:kx + P]
                                nc.tensor.matmul(
                                    pm[:, h, 0:HALF], lhsT=lhsT, rhs=rhs,
                                    start=(t == 0), stop=(t == KS * KS - 1),
                                    skip_group_check=True)

                        nc.scalar.activation(
                            out=osb[:, gl * PIX:(gl + 1) * PIX],
                            in_=pm[:, :, 0:HALF],
                            func=mybir.ActivationFunctionType.Copy,
                            bias=0.0, scale=r2[:, g:g + 1])
                    nc.gpsimd.dma_start(
                        out=outd[:, p0:p0 + npair].rearrange("p g f -> p (g f)"),
                        in_=osb[:, :npair * PIX])

    nc.compile()
    return nc


def _host_prep(guidance, depth, conv_w, conv_b, dense_w, dense_b):
    B, H, W, _ = guidance.shape
    nh, nw = H // P, W // P
    NB = B * nh * nw

    def to_samples(x):
        # (B,H,W,F) -> (NB, P, P, F), sample order = flat (b, i, j)
        return (x.reshape(B, nh, P, nw, P, F)
                 .transpose(0, 1, 3, 2, 4, 5)
                 .reshape(NB, P, P, F))

    gs = to_samples(np.ascontiguousarray(guidance))
    ds = to_samples(np.ascontiguousarray(depth))

    in_maps = []
    for c in range(NCORES):
        gsl = gs[c * 512:(c + 1) * 512]
        dsl = ds[c * 512:(c + 1) * 512]
        gpad = np.zeros((SPC, P, P, F), np.float32)
        gpad[:512] = gsl
        dpad = np.zeros((SPC, PADW, PADW, F), np.float32)
        dpad[:512, 1:P + 1, 1:P + 1] = dsl
        # (SPC, y, x, ch) -> [NGROUP, 126, pix]  with q = n_local*9 + ch
        gq = (gpad.reshape(NGROUP, NL, P, P, F)
                  .transpose(1, 4, 0, 2, 3)
                  .reshape(Q, NGROUP, PIX))
        dq = (dpad.reshape(NGROUP, NL, PADW, PADW, F)
                  .transpose(1, 4, 0, 2, 3)
                  .reshape(Q, NGROUP, PPIX))
        in_maps.append({"gin": np.ascontiguousarray(gq).astype(BF16),
                        "din": np.ascontiguousarray(dq).astype(BF16)})

    eye = np.eye(NL, dtype=np.float32)
    lhsA = np.zeros((KS * KS, Q, Q), np.float32)
    for t in range(KS * KS):
        ky, kx = divmod(t, KS)
        lhsA[t] = np.kron(eye, conv_w[ky, kx])
    lhsA = np.ascontiguousarray(lhsA.transpose(1, 0, 2))      # [Q, 9, Q]
    lhsD = np.zeros((F, Q + 1, Q), np.float32)
    lhsD2 = np.zeros((F, Q + 1, Q), np.float32)
    dws = dense_w.astype(np.float32) / PIX  # gap arrives as a SUM over pixels
    for j in range(F):
        lhsD[j, :Q] = np.kron(eye, dws[:, j * F:(j + 1) * F])
        lhsD[j, Q] = np.tile(dense_b[j * F:(j + 1) * F], NL)
        lhsD2[j, :Q] = np.kron(eye, dws[:, j::F])
        lhsD2[j, Q] = np.tile(dense_b[j::F], NL)
    lhsD = np.ascontiguousarray(lhsD.transpose(1, 0, 2))      # [Q+1, 9, Q]
    lhsD2 = np.ascontiguousarray(lhsD2.transpose(1, 0, 2))
    mask = np.kron(eye, np.ones((F, F), np.float32))
    convb = np.tile(conv_b.astype(np.float32), NL)[:, None]

    cpack = np.zeros((Q + 1, 3 * F * Q + Q), np.float32)
    cpack[0:Q, 0:F * Q] = lhsA.reshape(Q, F * Q)
    cpack[:, F * Q:2 * F * Q] = lhsD.reshape(Q + 1, F * Q)
    cpack[:, 2 * F * Q:3 * F * Q] = lhsD2.reshape(Q + 1, F * Q)
    cpack[0:Q, 3 * F * Q:3 * F * Q + Q] = mask
    consts = {"cpack": cpack.astype(BF16),
              "convb": np.ascontiguousarray(convb)}
    for m in in_maps:
        m.update(consts)
    return in_maps


_CACHED_NC = None


def run(inputs, trace=False, **kw):
    """Build (cached), run on 8 cores, return (full_output, BassKernelResults)."""
    global _CACHED_NC
    inputs = {k: np.asarray(v, np.float32) for k, v in inputs.items()}
    in_maps = _host_prep(**inputs)
    if _CACHED_NC is None:
        _CACHED_NC = build_program()
    res = run_bass_kernel_spmd(_CACHED_NC, in_maps, list(range(NCORES)),
                               trace=trace, **kw)
    outs = []
    for c in range(NCORES):
        o = res.results[c]["out"].astype(np.float32).reshape(NL, F, NGROUP, P, P)
        o = o.transpose(2, 0, 3, 4, 1).reshape(SPC, P, P, F)[:512]
        outs.append(o)
    full = np.concatenate(outs, 0)  # (4096, 24, 24, 9) in (b, i, j) order
    B, H, W = 16, 384, 384
    return full.reshape(B, H, W, F), res


def kernel(**inputs):
    out, _ = run(inputs, trace=False)
    return out



# revision 36
# speedup vs baseline: 1.2079x; 1.2079x over previous
"""Trainium2 Bass kernel for the Guided-Conv problem.

Math (per independent sample n, of NB = 4096):
  g_n, d_n : 24x24x9 patches of guidance / depth.
  c_n      = conv2d(g_n, conv_w, stride 8, SAME) + conv_b        -> 3x3x9
  k_n[i]   = c_n[:, :, i] / max(||c_n[:, :, i]||_2, 1)           (per-channel 3x3 filter)
  gap_n    = mean(g_n, (y, x))                                   -> 9
  W2_n     = (gap_n @ dense_w + dense_b).reshape(9, 9)           (i2 -> o2)
  r2_n[o]  = 1 / max(||W2_n[:, o]||_2, 1)
  out_n    = (depthwise(d_n, k_n) @ W2_n) * r2_n                 -> 24x24x9

Device strategy (per core: 512 samples + 6 pad = 37 groups of 14):
  Partition layout q = n_local*9 + ch on 126 partitions; free = pixels.
  - Kernel generation (c_n, W2_n) via block-diagonal matmuls: K = (n,ch),
    lhsT = kron(eye(14), w) built on host, so 14 samples per matmul.
  - Depthwise(3x3) + 1x1 fused: out[(n,o), pix] = sum_{t,i} BD_t[(n,i),(n,o)]
    * d_pad[(n,i), pix+t], 9 tap-matmuls accumulating in PSUM, float32r.
    BD_t = mask (.) (W2row-bcast) (.) k[:, t]  -- one DVE scalar_tensor_tensor.
  - r2 applied for free as the per-partition ACT scale on the PSUM->SBUF copy.
Host does all layout (patch extraction, channel de-interleave, zero-pad) --
this keeps every DMA contiguous in >=2KB runs.
"""

import numpy as np
import ml_dtypes

import concourse.bass as bass
from concourse import bacc
import concourse.mybir as mybir
from concourse.tile import TileContext
from concourse.bass_utils import run_bass_kernel_spmd

BF16 = ml_dtypes.bfloat16

F = 9          # channels
P = 24         # patch size
PADW = 26      # padded patch width (SAME conv, pad 1)
KS = 3         # generated kernel size
NCORES = 8
NL = 14        # samples per group
Q = NL * F     # 126 used partitions
NGROUP = 37    # groups per core (36 full + 1 padded)
SPC = NGROUP * NL  # 518 sample slots per core (512 real)
PIX = P * P        # 576
PPIX = PADW * PADW  # 676
HALF = PIX // 2    # 288, pixels per PSUM chunk (<=512 fp32/bank)
SUPER = [2, 7, 9, 9, 10]  # weight-gen supertile sizes (sum = 37);
                          # tiny first tile -> main conv starts early

F32 = mybir.dt.float32
BF = mybir.dt.bfloat16


def build_program():
    nc = bacc.Bacc("TRN2", target_bir_lowering=False, debug=False,
                   num_devices=NCORES)

    gin = nc.dram_tensor("gin", [Q, NGROUP, PIX], BF, kind="ExternalInput").ap()
    din = nc.dram_tensor("din", [Q, NGROUP, PPIX], BF, kind="ExternalInput").ap()
    lhsA = nc.dram_tensor("lhsA", [Q, KS * KS, Q], BF, kind="ExternalInput").ap()
    lhsD = nc.dram_tensor("lhsD", [Q + 1, F, Q], BF, kind="ExternalInput").ap()
    lhsD2 = nc.dram_tensor("lhsD2", [Q + 1, F, Q], BF, kind="ExternalInput").ap()
    maskd = nc.dram_tensor("mask", [Q, Q], BF, kind="ExternalInput").ap()
    convbd = nc.dram_tensor("convb", [Q, 1], F32, kind="ExternalInput").ap()
    outd = nc.dram_tensor("out", [Q, NGROUP, PIX], BF, kind="ExternalOutput").ap()

    with TileContext(nc) as tc:
        with (
            nc.allow_low_precision(reason="bf16 pipeline; tol 2e-2"),
            tc.tile_pool(name="consts", bufs=1) as cpool,
            tc.tile_pool(name="gpool", bufs=2) as gpool,
            tc.tile_pool(name="dpool", bufs=8) as dpool,
            tc.tile_pool(name="opool", bufs=6) as opool,
            tc.tile_pool(name="scrap", bufs=2) as scpool,
            tc.tile_pool(name="small", bufs=1) as spool,
            tc.tile_pool(name="bd", bufs=12) as bdpool,
            tc.tile_pool(name="ps_c", bufs=1, space="PSUM") as pcpool,
            tc.tile_pool(name="ps_d", bufs=1, space="PSUM") as pdpool,
            tc.tile_pool(name="ps_main", bufs=3, space="PSUM") as pmpool,
        ):
            # ---- constants (sync queue = HW descriptor-gen) ----
            lhsA_sb = cpool.tile([Q, KS * KS, Q], BF, tag="lhsA")
            nc.sync.dma_start(out=lhsA_sb, in_=lhsA)
            lhsD_sb = cpool.tile([Q + 1, F, Q], BF, tag="lhsD")
            nc.sync.dma_start(out=lhsD_sb, in_=lhsD)
            lhsD2_sb = cpool.tile([Q + 1, F, Q], BF, tag="lhsD2")
            nc.sync.dma_start(out=lhsD2_sb, in_=lhsD2)
            mask_sb = cpool.tile([Q, NL, F], BF, tag="mask")
            nc.sync.dma_start(out=mask_sb,
                              in_=maskd.rearrange("p (a b) -> p a b", b=F))
            convb_sb = cpool.tile([Q, 1], F32, tag="convb")
            nc.gpsimd.dma_start(out=convb_sb, in_=convbd)

            # ---- persistent per-core small tensors ----
            craw = spool.tile([Q, NGROUP, F], F32, tag="craw")     # c + conv_b
            knorm = spool.tile([Q, NGROUP, F], BF, tag="knorm")    # normalized taps
            w2 = spool.tile([Q, NGROUP, F], BF, tag="w2")          # raw W2 (D2 layout)
            r2 = spool.tile([Q, NGROUP], F32, tag="r2")            # 1/max(n2,1)
            r1 = spool.tile([Q, NGROUP], F32, tag="r1")            # 1/max(n1,1)
            sq = spool.tile([Q, NGROUP, F], F32, tag="sq")         # scratch squares
            s1 = spool.tile([Q, NGROUP], F32, tag="s1")            # scratch sums

            starts = []
            _g = 0
            for ng in SUPER:
                starts.append(_g)
                _g += ng

            pre = {}

            def emit_gin(si):
                # prefetch guidance for supertile si + its gap reduction
                ngi = SUPER[si]
                gsli = slice(starts[si], starts[si] + ngi)
                gsb = gpool.tile([Q, ngi * PIX], BF, tag="gsb")
                nc.sync.dma_start(out=gsb,
                                  in_=gin[:, gsli].rearrange("p g f -> p (g f)"))
                # gap: per-group pixel SUM (the 1/576 mean scale is folded
                # into lhsD/lhsD2 on the host). Row 126 must read 1.0 in the
                # K=127 dense matmuls, so memset the whole tile first.
                gap = spool.tile([128, ngi], BF, tag="gap", bufs=2)
                nc.vector.memset(gap, 1.0)
                nc.vector.tensor_reduce(
                    out=gap[0:Q, :],
                    in_=gsb.rearrange("p (g f) -> p g f", g=ngi),
                    axis=mybir.AxisListType.X, op=mybir.AluOpType.add)
                pre[si] = (gsb, gap)

            emit_gin(0)
            for si, ng in enumerate(SUPER):
                g0 = starts[si]
                gsl = slice(g0, g0 + ng)
                gsb, gap = pre.pop(si)

                # step A: strided conv -> c, 9 accumulated BD matmuls
                psc = pcpool.tile([Q, ng, F], F32, tag="psc")
                gwin = gsb.rearrange(
                    "p (g oy yr ox xr) -> p g oy ox yr xr",
                    g=ng, oy=KS, yr=8, ox=KS, xr=8)
                for t in range(KS * KS):
                    ky, kx = divmod(t, KS)
                    nc.tensor.matmul(
                        psc,
                        lhsT=lhsA_sb[:, t, :],
                        rhs=gwin[:, :, :, :, ky, kx],
                        start=(t == 0), stop=(t == KS * KS - 1),
                        skip_group_check=True)

                # craw = psc + conv_b (per-partition bias)
                nc.scalar.activation(
                    out=craw[:, gsl, :], in_=psc,
                    func=mybir.ActivationFunctionType.Identity,
                    bias=convb_sb, scale=1.0)

                # dense layer, both layouts (D for the norm, D2 for the values)
                psDall = pdpool.tile([Q, 2, F, ng], F32, tag="psDall")
                psD = psDall[:, 0]
                psD2 = psDall[:, 1]
                for j in range(F):
                    nc.tensor.matmul(psD[:, j, :], lhsT=lhsD_sb[:, j, :],
                                     rhs=gap[0:Q + 1, :],
                                     start=True, stop=True,
                                     skip_group_check=True)
                for j in range(F):
                    nc.tensor.matmul(psD2[:, j, :], lhsT=lhsD2_sb[:, j, :],
                                     rhs=gap[0:Q + 1, :],
                                     start=True, stop=True,
                                     skip_group_check=True)

                # r2 = 1/max(||W2[:,o]||, 1):  sum_i2 D^2 per (n,o2)
                nc.scalar.square(out=sq[:, gsl, :],
                                 in_=psD.rearrange("p i g -> p g i"))
                nc.vector.tensor_reduce(
                    out=s1[:, gsl], in_=sq[:, gsl, :],
                    axis=mybir.AxisListType.X, op=mybir.AluOpType.add)
                nc.scalar.sqrt(out=s1[:, gsl], in_=s1[:, gsl])
                nc.vector.tensor_scalar_max(r2[:, gsl], s1[:, gsl], 1.0)
                nc.vector.reciprocal(r2[:, gsl], r2[:, gsl])

                # W2 raw values, group-major
                nc.scalar.copy(out=w2[:, gsl, :],
                               in_=psD2.rearrange("p o g -> p g o"))

                # r1 = 1/max(||c||, 1) per (n, ch); knorm = craw * r1
                nc.scalar.square(out=sq[:, gsl, :], in_=craw[:, gsl, :])
                nc.vector.tensor_reduce(
                    out=s1[:, gsl], in_=sq[:, gsl, :],
                    axis=mybir.AxisListType.X, op=mybir.AluOpType.add)
                nc.scalar.sqrt(out=s1[:, gsl], in_=s1[:, gsl])
                nc.vector.tensor_scalar_max(r1[:, gsl], s1[:, gsl], 1.0)
                nc.vector.reciprocal(r1[:, gsl], r1[:, gsl])
                nc.vector.tensor_mul(
                    out=knorm[:, gsl, :], in0=craw[:, gsl, :],
                    in1=r1[:, gsl].unsqueeze(2).broadcast_to([Q, ng, F]))

                # ---------- main conv, groups in pairs (fewer DMA setups) ----
                npairs = (ng + 1) // 2
                for pi, p0 in enumerate(range(g0, g0 + ng, 2)):
                    npair = min(2, g0 + ng - p0)
                    dsb = dpool.tile([Q, 2 * PPIX], BF, tag="dsb")
                    nc.sync.dma_start(
                        out=dsb[:, :npair * PPIX],
                        in_=din[:, p0:p0 + npair].rearrange("p g f -> p (g f)"))
                    osb = opool.tile([Q, 2 * PIX], BF, tag="osb")

                    # W2 block-diagonal base for the pair: bdw = mask (.) w2
                    # (one DVE tensor_tensor, 2x mode), then per-tap scaling
                    # bd_t = bdw * k_t via tensor_scalar (4x) spread across
                    # DVE / ACT / GpSimd to balance engine load.
                    bdw = bdpool.tile([Q, 2, NL, F], BF, tag="bdw")
                    nc.vector.tensor_tensor(
                        out=bdw[:, :npair],
                        in0=mask_sb.unsqueeze(1).broadcast_to([Q, npair, NL, F]),
                        in1=w2[:, p0:p0 + npair, :].unsqueeze(2)
                            .broadcast_to([Q, npair, NL, F]),
                        op=mybir.AluOpType.mult)

                    for gl in range(npair):
                        g = p0 + gl
                        drows = dsb[:, gl * PPIX:(gl + 1) * PPIX].rearrange(
                            "p (r c) -> p r c", c=PADW)
                        # bd_t = bdw * k_t, batched: taps 0-5 in one DVE
                        # tensor_tensor, taps 6-8 on GpSimd.
                        bda = bdpool.tile([Q, 6, NL, F], BF, tag="bda")
                        nc.vector.tensor_tensor(
                            out=bda,
                            in0=bdw[:, gl].unsqueeze(1)
                                .broadcast_to([Q, 6, NL, F]),
                            in1=knorm[:, g, 0:6].unsqueeze(2).unsqueeze(3)
                                .broadcast_to([Q, 6, NL, F]),
                            op=mybir.AluOpType.mult)
                        bdb = bdpool.tile([Q, 3, NL, F], BF, tag="bdb")
                        nc.gpsimd.tensor_tensor(
                            out=bdb,
                            in0=bdw[:, gl].unsqueeze(1)
                                .broadcast_to([Q, 3, NL, F]),
                            in1=knorm[:, g, 6:9].unsqueeze(2).unsqueeze(3)
                                .broadcast_to([Q, 3, NL, F]),
                            op=mybir.AluOpType.mult)

                        pm = pmpool.tile([Q, 2, 512], F32, tag="pm")
                        for t in range(KS * KS):
                            ky, kx = divmod(t, KS)
                            src = bda[:, t] if t < 6 else bdb[:, t - 6]
                            lhsT = src.rearrange("p a b -> p (a b)")
                            for h in range(2):
                                rhs = drows[:, h * 12 + ky:h * 12 + ky + 12,
                                            kx:kx + P]
                                nc.tensor.matmul(
                                    pm[:, h, 0:HALF], lhsT=lhsT, rhs=rhs,
                                    start=(t == 0), stop=(t == KS * KS - 1),
                                    skip_group_check=True)

                        nc.scalar.activation(
                            out=osb[:, gl * PIX:(gl + 1) * PIX],
                            in_=pm[:, :, 0:HALF],
                            func=mybir.ActivationFunctionType.Copy,
                            bias=0.0, scale=r2[:, g:g + 1])
                    nc.gpsimd.dma_start(
                        out=outd[:, p0:p0 + npair].rearrange("p g f -> p (g f)"),
                        in_=osb[:, :npair * PIX])
                    if pi == min(1, npairs - 1) and si + 1 < len(SUPER):
                        emit_gin(si + 1)

    nc.compile()
    return nc


def _host_prep(guidance, depth, conv_w, conv_b, dense_w, dense_b):
    B, H, W, _ = guidance.shape
    nh, nw = H // P, W // P
    NB = B * nh * nw

    def to_samples(x):
        # (B,H,W,F) -> (NB, P, P, F), sample order = flat (b, i, j)
        return (x.reshape(B, nh, P, nw, P, F)
                 .transpose(0, 1, 3, 2, 4, 5)
                 .reshape(NB, P, P, F))

    gs = to_samples(np.ascontiguousarray(guidance))
    ds = to_samples(np.ascontiguousarray(depth))

    in_maps = []
    for c in range(NCORES):
        gsl = gs[c * 512:(c + 1) * 512]
        dsl = ds[c * 512:(c + 1) * 512]
        gpad = np.zeros((SPC, P, P, F), np.float32)
        gpad[:512] = gsl
        dpad = np.zeros((SPC, PADW, PADW, F), np.float32)
        dpad[:512, 1:P + 1, 1:P + 1] = dsl
        # (SPC, y, x, ch) -> [NGROUP, 126, pix]  with q = n_local*9 + ch
        gq = (gpad.reshape(NGROUP, NL, P, P, F)
                  .transpose(1, 4, 0, 2, 3)
                  .reshape(Q, NGROUP, PIX))
        dq = (dpad.reshape(NGROUP, NL, PADW, PADW, F)
                  .transpose(1, 4, 0, 2, 3)
                  .reshape(Q, NGROUP, PPIX))
        in_maps.append({"gin": np.ascontiguousarray(gq).astype(BF16),
                        "din": np.ascontiguousarray(dq).astype(BF16)})

    eye = np.eye(NL, dtype=np.float32)
    lhsA = np.zeros((KS * KS, Q, Q), np.float32)
    for t in range(KS * KS):
        ky, kx = divmod(t, KS)
        lhsA[t] = np.kron(eye, conv_w[ky, kx])
    lhsA = np.ascontiguousarray(lhsA.transpose(1, 0, 2))      # [Q, 9, Q]
    lhsD = np.zeros((F, Q + 1, Q), np.float32)
    lhsD2 = np.zeros((F, Q + 1, Q), np.float32)
    dws = dense_w.astype(np.float32) / PIX  # gap arrives as a SUM over pixels
    for j in range(F):
        lhsD[j, :Q] = np.kron(eye, dws[:, j * F:(j + 1) * F])
        lhsD[j, Q] = np.tile(dense_b[j * F:(j + 1) * F], NL)
        lhsD2[j, :Q] = np.kron(eye, dws[:, j::F])
        lhsD2[j, Q] = np.tile(dense_b[j::F], NL)
    lhsD = np.ascontiguousarray(lhsD.transpose(1, 0, 2))      # [Q+1, 9, Q]
    lhsD2 = np.ascontiguousarray(lhsD2.transpose(1, 0, 2))
    mask = np.kron(eye, np.ones((F, F), np.float32))
    convb = np.tile(conv_b.astype(np.float32), NL)[:, None]

    consts = {"lhsA": lhsA.astype(BF16), "lhsD": lhsD.astype(BF16),
              "lhsD2": lhsD2.astype(BF16),
              "mask": np.ascontiguousarray(mask).astype(BF16),
              "convb": np.ascontiguousarray(convb)}
    for m in in_maps:
        m.update(consts)
    return in_maps


_CACHED_NC = None


def run(inputs, trace=False, **kw):
    """Build (cached), run on 8 cores, return (full_output, BassKernelResults)."""
    global _CACHED_NC
    inputs = {k: np.asarray(v, np.float32) for k, v in inputs.items()}
    in_maps = _host_prep(**inputs)
    if _CACHED_NC is None:
        _CACHED_NC = build_program()
    res = run_bass_kernel_spmd(_CACHED_NC, in_maps, list(range(NCORES)),
                               trace=trace, **kw)
    outs = []
    for c in range(NCORES):
        o = res.results[c]["out"].astype(np.float32).reshape(NL, F, NGROUP, P, P)
        o = o.transpose(2, 0, 3, 4, 1).reshape(SPC, P, P, F)[:512]
        outs.append(o)
    full = np.concatenate(outs, 0)  # (4096, 24, 24, 9) in (b, i, j) order
    B, H, W = 16, 384, 384
    return full.reshape(B, H, W, F), res


def kernel(**inputs):
    out, _ = run(inputs, trace=False)
    return out



# revision 41
# speedup vs baseline: 1.3095x; 1.0841x over previous
"""Trainium2 Bass kernel for the Guided-Conv problem.

Math (per independent sample n, of NB = 4096):
  g_n, d_n : 24x24x9 patches of guidance / depth.
  c_n      = conv2d(g_n, conv_w, stride 8, SAME) + conv_b        -> 3x3x9
  k_n[i]   = c_n[:, :, i] / max(||c_n[:, :, i]||_2, 1)           (per-channel 3x3 filter)
  gap_n    = mean(g_n, (y, x))                                   -> 9
  W2_n     = (gap_n @ dense_w + dense_b).reshape(9, 9)           (i2 -> o2)
  r2_n[o]  = 1 / max(||W2_n[:, o]||_2, 1)
  out_n    = (depthwise(d_n, k_n) @ W2_n) * r2_n                 -> 24x24x9

Device strategy (per core: 512 samples + 6 pad = 37 groups of 14):
  Partition layout q = n_local*9 + ch on 126 partitions; free = pixels.
  - Kernel generation (c_n, W2_n) via block-diagonal matmuls: K = (n,ch),
    lhsT = kron(eye(14), w) built on host, so 14 samples per matmul.
  - Depthwise(3x3) + 1x1 fused: out[(n,o), pix] = sum_{t,i} BD_t[(n,i),(n,o)]
    * d_pad[(n,i), pix+t], 9 tap-matmuls accumulating in PSUM, float32r.
    BD_t = mask (.) (W2row-bcast) (.) k[:, t]  -- one DVE scalar_tensor_tensor.
  - r2 applied for free as the per-partition ACT scale on the PSUM->SBUF copy.
Host does all layout (patch extraction, channel de-interleave, zero-pad) --
this keeps every DMA contiguous in >=2KB runs.
"""

import numpy as np
import ml_dtypes

import concourse.bass as bass
from concourse import bacc
import concourse.mybir as mybir
from concourse.tile import TileContext
from concourse.bass_utils import run_bass_kernel_spmd

BF16 = ml_dtypes.bfloat16

F = 9          # channels
P = 24         # patch size
PADW = 26      # padded patch width (SAME conv, pad 1)
KS = 3         # generated kernel size
NCORES = 8
NL = 14        # samples per group
Q = NL * F     # 126 used partitions
NGROUP = 37    # groups per core (36 full + 1 padded)
SPC = NGROUP * NL  # 518 sample slots per core (512 real)
PIX = P * P        # 576
PPIX = PADW * PADW  # 676
HALF = PIX // 2    # 288, pixels per PSUM chunk (<=512 fp32/bank)
SUPER = [4, 8, 8, 8, 9]   # weight-gen supertile sizes (sum = 37);
                          # small first tile -> main conv starts early

F32 = mybir.dt.float32
BF = mybir.dt.bfloat16


def build_program():
    nc = bacc.Bacc("TRN2", target_bir_lowering=False, debug=False,
                   num_devices=NCORES)

    gin = nc.dram_tensor("gin", [Q, NGROUP, PIX], BF, kind="ExternalInput").ap()
    din = nc.dram_tensor("din", [Q, NGROUP, PPIX], BF, kind="ExternalInput").ap()
    lhsA = nc.dram_tensor("lhsA", [Q, KS * KS, Q], BF, kind="ExternalInput").ap()
    lhsD = nc.dram_tensor("lhsD", [Q + 1, F, Q], BF, kind="ExternalInput").ap()
    lhsD2 = nc.dram_tensor("lhsD2", [Q + 1, F, Q], BF, kind="ExternalInput").ap()
    maskd = nc.dram_tensor("mask", [Q, Q], BF, kind="ExternalInput").ap()
    convbd = nc.dram_tensor("convb", [Q, 1], F32, kind="ExternalInput").ap()
    outd = nc.dram_tensor("out", [Q, NGROUP, PIX], BF, kind="ExternalOutput").ap()

    with TileContext(nc) as tc:
        with (
            nc.allow_low_precision(reason="bf16 pipeline; tol 2e-2"),
            tc.tile_pool(name="consts", bufs=1) as cpool,
            tc.tile_pool(name="gpool", bufs=2) as gpool,
            tc.tile_pool(name="dpool", bufs=8) as dpool,
            tc.tile_pool(name="opool", bufs=6) as opool,
            tc.tile_pool(name="scrap", bufs=2) as scpool,
            tc.tile_pool(name="small", bufs=1) as spool,
            tc.tile_pool(name="bd", bufs=12) as bdpool,
            tc.tile_pool(name="ps_c", bufs=1, space="PSUM") as pcpool,
            tc.tile_pool(name="ps_d", bufs=1, space="PSUM") as pdpool,
            tc.tile_pool(name="ps_main", bufs=3, space="PSUM") as pmpool,
        ):
            # ---- constants ----
            # Fully-contiguous DRAM sources lower to ONE DMA descriptor,
            # which lands on a single DMA engine (~22 GB/s). Chunk by
            # partition range so the transfers spread across engines.
            def dma_const(tile_ap, dram_ap, nchunks):
                pn = tile_ap.shape[0]
                step = (pn + nchunks - 1) // nchunks
                for i in range(0, pn, step):
                    j = min(pn, i + step)
                    nc.sync.dma_start(out=tile_ap[i:j], in_=dram_ap[i:j])

            lhsA_sb = cpool.tile([Q, KS * KS, Q], BF, tag="lhsA")
            lhsD_sb = cpool.tile([Q + 1, F, Q], BF, tag="lhsD")
            lhsD2_sb = cpool.tile([Q + 1, F, Q], BF, tag="lhsD2")
            mask_sb = cpool.tile([Q, NL, F], BF, tag="mask")
            convb_sb = cpool.tile([Q, 1], F32, tag="convb")

            # ---- persistent per-core small tensors ----
            craw = spool.tile([Q, NGROUP, F], F32, tag="craw")     # c + conv_b
            knorm = spool.tile([Q, NGROUP, F], BF, tag="knorm")    # normalized taps
            w2 = spool.tile([Q, NGROUP, F], BF, tag="w2")          # raw W2 (D2 layout)
            r2 = spool.tile([Q, NGROUP], F32, tag="r2")            # 1/max(n2,1)
            r1 = spool.tile([Q, NGROUP], F32, tag="r1")            # 1/max(n1,1)
            sq = spool.tile([Q, NGROUP, F], F32, tag="sq")         # scratch squares
            s1 = spool.tile([Q, NGROUP], F32, tag="s1")            # scratch sums

            starts = []
            _g = 0
            for ng in SUPER:
                starts.append(_g)
                _g += ng

            pre = {}

            def emit_gin(si):
                # prefetch guidance for supertile si + its gap reduction
                ngi = SUPER[si]
                gsli = slice(starts[si], starts[si] + ngi)
                gsb = gpool.tile([Q, ngi * PIX], BF, tag="gsb")
                nc.sync.dma_start(out=gsb,
                                  in_=gin[:, gsli].rearrange("p g f -> p (g f)"))
                # gap: per-group pixel SUM (the 1/576 mean scale is folded
                # into lhsD/lhsD2 on the host). Row 126 must read 1.0 in the
                # K=127 dense matmuls, so memset the whole tile first.
                gap = spool.tile([128, ngi], BF, tag="gap", bufs=2)
                nc.vector.memset(gap, 1.0)
                nc.vector.tensor_reduce(
                    out=gap[0:Q, :],
                    in_=gsb.rearrange("p (g f) -> p g f", g=ngi),
                    axis=mybir.AxisListType.X, op=mybir.AluOpType.add)
                pre[si] = (gsb, gap)

            dpre = {}

            def emit_din(p0, npair):
                dsb = dpool.tile([Q, 2 * PPIX], BF, tag="dsb")
                nc.sync.dma_start(
                    out=dsb[:, :npair * PPIX],
                    in_=din[:, p0:p0 + npair].rearrange("p g f -> p (g f)"))
                dpre[p0] = dsb

            # startup order: guidance + first depth pair first (they gate
            # compute), then the constant chunks, all interleaving on the
            # DMA engines.
            emit_gin(0)
            emit_din(0, 2)
            dma_const(lhsA_sb, lhsA, 4)
            dma_const(mask_sb, maskd.rearrange("p (a b) -> p a b", b=F), 2)
            dma_const(lhsD_sb, lhsD, 4)
            dma_const(lhsD2_sb, lhsD2, 4)
            nc.gpsimd.dma_start(out=convb_sb, in_=convbd)

            for si, ng in enumerate(SUPER):
                g0 = starts[si]
                gsl = slice(g0, g0 + ng)
                gsb, gap = pre.pop(si)

                # step A: strided conv -> c, 9 accumulated BD matmuls
                psc = pcpool.tile([Q, ng, F], F32, tag="psc")
                gwin = gsb.rearrange(
                    "p (g oy yr ox xr) -> p g oy ox yr xr",
                    g=ng, oy=KS, yr=8, ox=KS, xr=8)
                for t in range(KS * KS):
                    ky, kx = divmod(t, KS)
                    nc.tensor.matmul(
                        psc,
                        lhsT=lhsA_sb[:, t, :],
                        rhs=gwin[:, :, :, :, ky, kx],
                        start=(t == 0), stop=(t == KS * KS - 1),
                        skip_group_check=True)

                # craw = psc + conv_b (per-partition bias)
                nc.scalar.activation(
                    out=craw[:, gsl, :], in_=psc,
                    func=mybir.ActivationFunctionType.Identity,
                    bias=convb_sb, scale=1.0)

                # dense layer, both layouts (D for the norm, D2 for the values)
                psDall = pdpool.tile([Q, 2, F, ng], F32, tag="psDall")
                psD = psDall[:, 0]
                psD2 = psDall[:, 1]
                for j in range(F):
                    nc.tensor.matmul(psD[:, j, :], lhsT=lhsD_sb[:, j, :],
                                     rhs=gap[0:Q + 1, :],
                                     start=True, stop=True,
                                     skip_group_check=True)
                for j in range(F):
                    nc.tensor.matmul(psD2[:, j, :], lhsT=lhsD2_sb[:, j, :],
                                     rhs=gap[0:Q + 1, :],
                                     start=True, stop=True,
                                     skip_group_check=True)

                # r2 = 1/max(||W2[:,o]||, 1):  sum_i2 D^2 per (n,o2)
                nc.scalar.square(out=sq[:, gsl, :],
                                 in_=psD.rearrange("p i g -> p g i"))
                nc.vector.tensor_reduce(
                    out=s1[:, gsl], in_=sq[:, gsl, :],
                    axis=mybir.AxisListType.X, op=mybir.AluOpType.add)
                nc.scalar.sqrt(out=s1[:, gsl], in_=s1[:, gsl])
                nc.vector.tensor_scalar_max(r2[:, gsl], s1[:, gsl], 1.0)
                nc.vector.reciprocal(r2[:, gsl], r2[:, gsl])

                # W2 raw values, group-major
                nc.scalar.copy(out=w2[:, gsl, :],
                               in_=psD2.rearrange("p o g -> p g o"))

                # r1 = 1/max(||c||, 1) per (n, ch); knorm = craw * r1
                nc.scalar.square(out=sq[:, gsl, :], in_=craw[:, gsl, :])
                nc.vector.tensor_reduce(
                    out=s1[:, gsl], in_=sq[:, gsl, :],
                    axis=mybir.AxisListType.X, op=mybir.AluOpType.add)
                nc.scalar.sqrt(out=s1[:, gsl], in_=s1[:, gsl])
                nc.vector.tensor_scalar_max(r1[:, gsl], s1[:, gsl], 1.0)
                nc.vector.reciprocal(r1[:, gsl], r1[:, gsl])
                nc.vector.tensor_mul(
                    out=knorm[:, gsl, :], in0=craw[:, gsl, :],
                    in1=r1[:, gsl].unsqueeze(2).broadcast_to([Q, ng, F]))

                # ---------- main conv, groups in pairs (fewer DMA setups) ----
                npairs = (ng + 1) // 2
                for pi, p0 in enumerate(range(g0, g0 + ng, 2)):
                    npair = min(2, g0 + ng - p0)
                    dsb = dpre.pop(p0, None)
                    if dsb is None:
                        emit_din(p0, npair)
                        dsb = dpre.pop(p0)
                    osb = opool.tile([Q, 2 * PIX], BF, tag="osb")

                    # W2 block-diagonal base for the pair: bdw = mask (.) w2
                    # (one DVE tensor_tensor, 2x mode), then per-tap scaling
                    # bd_t = bdw * k_t via tensor_scalar (4x) spread across
                    # DVE / ACT / GpSimd to balance engine load.
                    bdw = bdpool.tile([Q, 2, NL, F], BF, tag="bdw")
                    nc.vector.tensor_tensor(
                        out=bdw[:, :npair],
                        in0=mask_sb.unsqueeze(1).broadcast_to([Q, npair, NL, F]),
                        in1=w2[:, p0:p0 + npair, :].unsqueeze(2)
                            .broadcast_to([Q, npair, NL, F]),
                        op=mybir.AluOpType.mult)

                    for gl in range(npair):
                        g = p0 + gl
                        drows = dsb[:, gl * PPIX:(gl + 1) * PPIX].rearrange(
                            "p (r c) -> p r c", c=PADW)
                        # bd_t = bdw * k_t, batched: taps 0-5 in one DVE
                        # tensor_tensor, taps 6-8 on GpSimd.
                        bda = bdpool.tile([Q, 6, NL, F], BF, tag="bda")
                        nc.vector.tensor_tensor(
                            out=bda,
                            in0=bdw[:, gl].unsqueeze(1)
                                .broadcast_to([Q, 6, NL, F]),
                            in1=knorm[:, g, 0:6].unsqueeze(2).unsqueeze(3)
                                .broadcast_to([Q, 6, NL, F]),
                            op=mybir.AluOpType.mult)
                        bdb = bdpool.tile([Q, 3, NL, F], BF, tag="bdb")
                        nc.gpsimd.tensor_tensor(
                            out=bdb,
                            in0=bdw[:, gl].unsqueeze(1)
                                .broadcast_to([Q, 3, NL, F]),
                            in1=knorm[:, g, 6:9].unsqueeze(2).unsqueeze(3)
                                .broadcast_to([Q, 3, NL, F]),
                            op=mybir.AluOpType.mult)

                        pm = pmpool.tile([Q, 2, 512], F32, tag="pm")
                        for t in range(KS * KS):
                            ky, kx = divmod(t, KS)
                            src = bda[:, t] if t < 6 else bdb[:, t - 6]
                            lhsT = src.rearrange("p a b -> p (a b)")
                            for h in range(2):
                                rhs = drows[:, h * 12 + ky:h * 12 + ky + 12,
                                            kx:kx + P]
                                nc.tensor.matmul(
                                    pm[:, h, 0:HALF], lhsT=lhsT, rhs=rhs,
                                    start=(t == 0), stop=(t == KS * KS - 1),
                                    skip_group_check=True)

                        nc.scalar.activation(
                            out=osb[:, gl * PIX:(gl + 1) * PIX],
                            in_=pm[:, :, 0:HALF],
                            func=mybir.ActivationFunctionType.Copy,
                            bias=0.0, scale=r2[:, g:g + 1])
                    nc.gpsimd.dma_start(
                        out=outd[:, p0:p0 + npair].rearrange("p g f -> p (g f)"),
                        in_=osb[:, :npair * PIX])
                    if pi == 0 and si + 1 < len(SUPER):
                        emit_gin(si + 1)

    nc.compile()
    return nc


def _host_prep(guidance, depth, conv_w, conv_b, dense_w, dense_b):
    B, H, W, _ = guidance.shape
    nh, nw = H // P, W // P
    NB = B * nh * nw

    def to_samples(x):
        # (B,H,W,F) -> (NB, P, P, F), sample order = flat (b, i, j)
        return (x.reshape(B, nh, P, nw, P, F)
                 .transpose(0, 1, 3, 2, 4, 5)
                 .reshape(NB, P, P, F))

    gs = to_samples(np.ascontiguousarray(guidance))
    ds = to_samples(np.ascontiguousarray(depth))

    in_maps = []
    for c in range(NCORES):
        gsl = gs[c * 512:(c + 1) * 512]
        dsl = ds[c * 512:(c + 1) * 512]
        gpad = np.zeros((SPC, P, P, F), np.float32)
        gpad[:512] = gsl
        dpad = np.zeros((SPC, PADW, PADW, F), np.float32)
        dpad[:512, 1:P + 1, 1:P + 1] = dsl
        # (SPC, y, x, ch) -> [NGROUP, 126, pix]  with q = n_local*9 + ch
        gq = (gpad.reshape(NGROUP, NL, P, P, F)
                  .transpose(1, 4, 0, 2, 3)
                  .reshape(Q, NGROUP, PIX))
        dq = (dpad.reshape(NGROUP, NL, PADW, PADW, F)
                  .transpose(1, 4, 0, 2, 3)
                  .reshape(Q, NGROUP, PPIX))
        in_maps.append({"gin": np.ascontiguousarray(gq).astype(BF16),
                        "din": np.ascontiguousarray(dq).astype(BF16)})

    eye = np.eye(NL, dtype=np.float32)
    lhsA = np.zeros((KS * KS, Q, Q), np.float32)
    for t in range(KS * KS):
        ky, kx = divmod(t, KS)
        lhsA[t] = np.kron(eye, conv_w[ky, kx])
    lhsA = np.ascontiguousarray(lhsA.transpose(1, 0, 2))      # [Q, 9, Q]
    lhsD = np.zeros((F, Q + 1, Q), np.float32)
    lhsD2 = np.zeros((F, Q + 1, Q), np.float32)
    dws = dense_w.astype(np.float32) / PIX  # gap arrives as a SUM over pixels
    for j in range(F):
        lhsD[j, :Q] = np.kron(eye, dws[:, j * F:(j + 1) * F])
        lhsD[j, Q] = np.tile(dense_b[j * F:(j + 1) * F], NL)
        lhsD2[j, :Q] = np.kron(eye, dws[:, j::F])
        lhsD2[j, Q] = np.tile(dense_b[j::F], NL)
    lhsD = np.ascontiguousarray(lhsD.transpose(1, 0, 2))      # [Q+1, 9, Q]
    lhsD2 = np.ascontiguousarray(lhsD2.transpose(1, 0, 2))
    mask = np.kron(eye, np.ones((F, F), np.float32))
    convb = np.tile(conv_b.astype(np.float32), NL)[:, None]

    consts = {"lhsA": lhsA.astype(BF16), "lhsD": lhsD.astype(BF16),
              "lhsD2": lhsD2.astype(BF16),
              "mask": np.ascontiguousarray(mask).astype(BF16),
              "convb": np.ascontiguousarray(convb)}
    for m in in_maps:
        m.update(consts)
    return in_maps


_CACHED_NC = None


def run(inputs, trace=False, **kw):
    """Build (cached), run on 8 cores, return (full_output, BassKernelResults)."""
    global _CACHED_NC
    inputs = {k: np.asarray(v, np.float32) for k, v in inputs.items()}
    in_maps = _host_prep(**inputs)
    if _CACHED_NC is None:
        _CACHED_NC = build_program()
    res = run_bass_kernel_spmd(_CACHED_NC, in_maps, list(range(NCORES)),
                               trace=trace, **kw)
    outs = []
    for c in range(NCORES):
        o = res.results[c]["out"].astype(np.float32).reshape(NL, F, NGROUP, P, P)
        o = o.transpose(2, 0, 3, 4, 1).reshape(SPC, P, P, F)[:512]
        outs.append(o)
    full = np.concatenate(outs, 0)  # (4096, 24, 24, 9) in (b, i, j) order
    B, H, W = 16, 384, 384
    return full.reshape(B, H, W, F), res


def kernel(**inputs):
    out, _ = run(inputs, trace=False)
    return out



# revision 43
# speedup vs baseline: 1.3912x; 1.0624x over previous
"""Trainium2 Bass kernel for the Guided-Conv problem.

Math (per independent sample n, of NB = 4096):
  g_n, d_n : 24x24x9 patches of guidance / depth.
  c_n      = conv2d(g_n, conv_w, stride 8, SAME) + conv_b        -> 3x3x9
  k_n[i]   = c_n[:, :, i] / max(||c_n[:, :, i]||_2, 1)           (per-channel 3x3 filter)
  gap_n    = mean(g_n, (y, x))                                   -> 9
  W2_n     = (gap_n @ dense_w + dense_b).reshape(9, 9)           (i2 -> o2)
  r2_n[o]  = 1 / max(||W2_n[:, o]||_2, 1)
  out_n    = (depthwise(d_n, k_n) @ W2_n) * r2_n                 -> 24x24x9

Device strategy (per core: 512 samples + 6 pad = 37 groups of 14):
  Partition layout q = n_local*9 + ch on 126 partitions; free = pixels.
  - Kernel generation (c_n, W2_n) via block-diagonal matmuls: K = (n,ch),
    lhsT = kron(eye(14), w) built on host, so 14 samples per matmul.
  - Depthwise(3x3) + 1x1 fused: out[(n,o), pix] = sum_{t,i} BD_t[(n,i),(n,o)]
    * d_pad[(n,i), pix+t], 9 tap-matmuls accumulating in PSUM, float32r.
    BD_t = mask (.) (W2row-bcast) (.) k[:, t]  -- one DVE scalar_tensor_tensor.
  - r2 applied for free as the per-partition ACT scale on the PSUM->SBUF copy.
Host does all layout (patch extraction, channel de-interleave, zero-pad) --
this keeps every DMA contiguous in >=2KB runs.
"""

import numpy as np
import ml_dtypes

import concourse.bass as bass
from concourse import bacc
import concourse.mybir as mybir
from concourse.tile import TileContext
from concourse.bass_utils import run_bass_kernel_spmd

BF16 = ml_dtypes.bfloat16

F = 9          # channels
P = 24         # patch size
PADW = 26      # padded patch width (SAME conv, pad 1)
KS = 3         # generated kernel size
NCORES = 8
NL = 14        # samples per group
Q = NL * F     # 126 used partitions
NGROUP = 37    # groups per core (36 full + 1 padded)
SPC = NGROUP * NL  # 518 sample slots per core (512 real)
PIX = P * P        # 576
PPIX = PADW * PADW  # 676
HALF = PIX // 2    # 288, pixels per PSUM chunk (<=512 fp32/bank)
SUPER = [4, 8, 8, 8, 9]   # weight-gen supertile sizes (sum = 37);
                          # small first tile -> main conv starts early

F32 = mybir.dt.float32
BF = mybir.dt.bfloat16


def build_program():
    nc = bacc.Bacc("TRN2", target_bir_lowering=False, debug=False,
                   num_devices=NCORES)

    gin = nc.dram_tensor("gin", [Q, NGROUP, PIX], BF, kind="ExternalInput").ap()
    din = nc.dram_tensor("din", [Q, NGROUP, PPIX], BF, kind="ExternalInput").ap()
    lhsA = nc.dram_tensor("lhsA", [Q, KS * KS, Q], BF, kind="ExternalInput").ap()
    lhsD = nc.dram_tensor("lhsD", [Q + 1, F, Q], BF, kind="ExternalInput").ap()
    lhsD2 = nc.dram_tensor("lhsD2", [Q + 1, F, Q], BF, kind="ExternalInput").ap()
    maskd = nc.dram_tensor("mask", [Q, Q], BF, kind="ExternalInput").ap()
    convbd = nc.dram_tensor("convb", [Q, 1], F32, kind="ExternalInput").ap()
    outd = nc.dram_tensor("out", [Q, NGROUP, PIX], BF, kind="ExternalOutput").ap()

    with TileContext(nc) as tc:
        with (
            nc.allow_low_precision(reason="bf16 pipeline; tol 2e-2"),
            tc.tile_pool(name="consts", bufs=1) as cpool,
            tc.tile_pool(name="gpool", bufs=2) as gpool,
            tc.tile_pool(name="dpool", bufs=8) as dpool,
            tc.tile_pool(name="opool", bufs=6) as opool,
            tc.tile_pool(name="scrap", bufs=2) as scpool,
            tc.tile_pool(name="small", bufs=1) as spool,
            tc.tile_pool(name="bd", bufs=12) as bdpool,
            tc.tile_pool(name="ps_c", bufs=1, space="PSUM") as pcpool,
            tc.tile_pool(name="ps_d", bufs=1, space="PSUM") as pdpool,
            tc.tile_pool(name="ps_main", bufs=3, space="PSUM") as pmpool,
        ):
            # ---- constants ----
            # Fully-contiguous DRAM sources lower to ONE DMA descriptor,
            # which lands on a single DMA engine (~22 GB/s). Chunk by
            # partition range so the transfers spread across engines.
            def dma_const(tile_ap, dram_ap, nchunks):
                pn = tile_ap.shape[0]
                step = (pn + nchunks - 1) // nchunks
                for i in range(0, pn, step):
                    j = min(pn, i + step)
                    nc.sync.dma_start(out=tile_ap[i:j], in_=dram_ap[i:j])

            lhsA_sb = cpool.tile([Q, KS * KS, Q], BF, tag="lhsA")
            lhsD_sb = cpool.tile([Q + 1, F, Q], BF, tag="lhsD")
            lhsD2_sb = cpool.tile([Q + 1, F, Q], BF, tag="lhsD2")
            mask_sb = cpool.tile([Q, NL, F], BF, tag="mask")
            convb_sb = cpool.tile([Q, 1], F32, tag="convb")

            # ---- persistent per-core small tensors ----
            craw = spool.tile([Q, NGROUP, F], F32, tag="craw")     # c + conv_b
            knorm = spool.tile([Q, NGROUP, F], BF, tag="knorm")    # normalized taps
            w2 = spool.tile([Q, NGROUP, F], BF, tag="w2")          # raw W2 (D2 layout)
            r2 = spool.tile([Q, NGROUP], F32, tag="r2")            # 1/max(n2,1)
            r1 = spool.tile([Q, NGROUP], F32, tag="r1")            # 1/max(n1,1)
            sq = spool.tile([Q, NGROUP, F], F32, tag="sq")         # scratch squares
            s1 = spool.tile([Q, NGROUP], F32, tag="s1")            # scratch sums

            starts = []
            _g = 0
            for ng in SUPER:
                starts.append(_g)
                _g += ng

            pre = {}

            def emit_gin(si):
                # prefetch guidance for supertile si + its gap reduction
                ngi = SUPER[si]
                gsli = slice(starts[si], starts[si] + ngi)
                gsb = gpool.tile([Q, ngi * PIX], BF, tag="gsb")
                nc.sync.dma_start(out=gsb,
                                  in_=gin[:, gsli].rearrange("p g f -> p (g f)"))
                # gap: per-group pixel SUM (the 1/576 mean scale is folded
                # into lhsD/lhsD2 on the host). Row 126 must read 1.0 in the
                # K=127 dense matmuls, so memset the whole tile first.
                gap = spool.tile([128, ngi], BF, tag="gap", bufs=2)
                nc.vector.memset(gap, 1.0)
                nc.vector.tensor_reduce(
                    out=gap[0:Q, :],
                    in_=gsb.rearrange("p (g f) -> p g f", g=ngi),
                    axis=mybir.AxisListType.X, op=mybir.AluOpType.add)
                pre[si] = (gsb, gap)

            dpre = {}

            def emit_din(p0, npair):
                dsb = dpool.tile([Q, 2 * PPIX], BF, tag="dsb")
                nc.sync.dma_start(
                    out=dsb[:, :npair * PPIX],
                    in_=din[:, p0:p0 + npair].rearrange("p g f -> p (g f)"))
                dpre[p0] = dsb

            # startup order: guidance + first depth pair first (they gate
            # compute), then the constant chunks, all interleaving on the
            # DMA engines.
            emit_gin(0)
            emit_din(0, 2)
            dma_const(lhsA_sb, lhsA, 4)
            dma_const(mask_sb, maskd.rearrange("p (a b) -> p a b", b=F), 2)
            dma_const(lhsD_sb, lhsD, 4)
            dma_const(lhsD2_sb, lhsD2, 4)
            nc.gpsimd.dma_start(out=convb_sb, in_=convbd)

            def emit_wg(si):
                # weight generation (KGL conv + dense + norms) for supertile si
                ng = SUPER[si]
                g0 = starts[si]
                gsl = slice(g0, g0 + ng)
                gsb, gap = pre.pop(si)

                # step A: strided conv -> c, 9 accumulated BD matmuls
                psc = pcpool.tile([Q, ng, F], F32, tag="psc")
                gwin = gsb.rearrange(
                    "p (g oy yr ox xr) -> p g oy ox yr xr",
                    g=ng, oy=KS, yr=8, ox=KS, xr=8)
                for t in range(KS * KS):
                    ky, kx = divmod(t, KS)
                    nc.tensor.matmul(
                        psc,
                        lhsT=lhsA_sb[:, t, :],
                        rhs=gwin[:, :, :, :, ky, kx],
                        start=(t == 0), stop=(t == KS * KS - 1),
                        skip_group_check=True)

                # craw = psc + conv_b (per-partition bias)
                nc.scalar.activation(
                    out=craw[:, gsl, :], in_=psc,
                    func=mybir.ActivationFunctionType.Identity,
                    bias=convb_sb, scale=1.0)

                # dense layer, both layouts (D for the norm, D2 for the values)
                psDall = pdpool.tile([Q, 2, F, ng], F32, tag="psDall")
                psD = psDall[:, 0]
                psD2 = psDall[:, 1]
                for j in range(F):
                    nc.tensor.matmul(psD[:, j, :], lhsT=lhsD_sb[:, j, :],
                                     rhs=gap[0:Q + 1, :],
                                     start=True, stop=True,
                                     skip_group_check=True)
                for j in range(F):
                    nc.tensor.matmul(psD2[:, j, :], lhsT=lhsD2_sb[:, j, :],
                                     rhs=gap[0:Q + 1, :],
                                     start=True, stop=True,
                                     skip_group_check=True)

                # r2 = 1/max(||W2[:,o]||, 1):  sum_i2 D^2 per (n,o2)
                nc.scalar.square(out=sq[:, gsl, :],
                                 in_=psD.rearrange("p i g -> p g i"))
                nc.vector.tensor_reduce(
                    out=s1[:, gsl], in_=sq[:, gsl, :],
                    axis=mybir.AxisListType.X, op=mybir.AluOpType.add)
                nc.scalar.sqrt(out=s1[:, gsl], in_=s1[:, gsl])
                nc.vector.tensor_scalar_max(r2[:, gsl], s1[:, gsl], 1.0)
                nc.vector.reciprocal(r2[:, gsl], r2[:, gsl])

                # W2 raw values, group-major
                nc.scalar.copy(out=w2[:, gsl, :],
                               in_=psD2.rearrange("p o g -> p g o"))

                # r1 = 1/max(||c||, 1) per (n, ch); knorm = craw * r1
                nc.scalar.square(out=sq[:, gsl, :], in_=craw[:, gsl, :])
                nc.vector.tensor_reduce(
                    out=s1[:, gsl], in_=sq[:, gsl, :],
                    axis=mybir.AxisListType.X, op=mybir.AluOpType.add)
                nc.scalar.sqrt(out=s1[:, gsl], in_=s1[:, gsl])
                nc.vector.tensor_scalar_max(r1[:, gsl], s1[:, gsl], 1.0)
                nc.vector.reciprocal(r1[:, gsl], r1[:, gsl])
                nc.vector.tensor_mul(
                    out=knorm[:, gsl, :], in0=craw[:, gsl, :],
                    in1=r1[:, gsl].unsqueeze(2).broadcast_to([Q, ng, F]))

            emit_wg(0)
            for si, ng in enumerate(SUPER):
                g0 = starts[si]

                # ---------- main conv, groups in pairs (fewer DMA setups) ----
                npairs = (ng + 1) // 2
                for pi, p0 in enumerate(range(g0, g0 + ng, 2)):
                    npair = min(2, g0 + ng - p0)
                    dsb = dpre.pop(p0, None)
                    if dsb is None:
                        emit_din(p0, npair)
                        dsb = dpre.pop(p0)
                    osb = opool.tile([Q, 2 * PIX], BF, tag="osb")

                    # W2 block-diagonal base for the pair: bdw = mask (.) w2
                    # (one DVE tensor_tensor, 2x mode), then per-tap scaling
                    # bd_t = bdw * k_t via tensor_scalar (4x) spread across
                    # DVE / ACT / GpSimd to balance engine load.
                    bdw = bdpool.tile([Q, 2, NL, F], BF, tag="bdw")
                    nc.vector.tensor_tensor(
                        out=bdw[:, :npair],
                        in0=mask_sb.unsqueeze(1).broadcast_to([Q, npair, NL, F]),
                        in1=w2[:, p0:p0 + npair, :].unsqueeze(2)
                            .broadcast_to([Q, npair, NL, F]),
                        op=mybir.AluOpType.mult)

                    for gl in range(npair):
                        g = p0 + gl
                        drows = dsb[:, gl * PPIX:(gl + 1) * PPIX].rearrange(
                            "p (r c) -> p r c", c=PADW)
                        # bd_t = bdw * k_t, batched: taps 0-5 in one DVE
                        # tensor_tensor, taps 6-8 on GpSimd.
                        bda = bdpool.tile([Q, 6, NL, F], BF, tag="bda")
                        nc.vector.tensor_tensor(
                            out=bda,
                            in0=bdw[:, gl].unsqueeze(1)
                                .broadcast_to([Q, 6, NL, F]),
                            in1=knorm[:, g, 0:6].unsqueeze(2).unsqueeze(3)
                                .broadcast_to([Q, 6, NL, F]),
                            op=mybir.AluOpType.mult)
                        bdb = bdpool.tile([Q, 3, NL, F], BF, tag="bdb")
                        nc.gpsimd.tensor_tensor(
                            out=bdb,
                            in0=bdw[:, gl].unsqueeze(1)
                                .broadcast_to([Q, 3, NL, F]),
                            in1=knorm[:, g, 6:9].unsqueeze(2).unsqueeze(3)
                                .broadcast_to([Q, 3, NL, F]),
                            op=mybir.AluOpType.mult)

                        pm = pmpool.tile([Q, 2, 512], F32, tag="pm")
                        for t in range(KS * KS):
                            ky, kx = divmod(t, KS)
                            src = bda[:, t] if t < 6 else bdb[:, t - 6]
                            lhsT = src.rearrange("p a b -> p (a b)")
                            for h in range(2):
                                rhs = drows[:, h * 12 + ky:h * 12 + ky + 12,
                                            kx:kx + P]
                                nc.tensor.matmul(
                                    pm[:, h, 0:HALF], lhsT=lhsT, rhs=rhs,
                                    start=(t == 0), stop=(t == KS * KS - 1),
                                    skip_group_check=True)

                        nc.scalar.activation(
                            out=osb[:, gl * PIX:(gl + 1) * PIX],
                            in_=pm[:, :, 0:HALF],
                            func=mybir.ActivationFunctionType.Copy,
                            bias=0.0, scale=r2[:, g:g + 1])
                    nc.gpsimd.dma_start(
                        out=outd[:, p0:p0 + npair].rearrange("p g f -> p (g f)"),
                        in_=osb[:, :npair * PIX])
                    if pi == 0 and si + 1 < len(SUPER):
                        emit_gin(si + 1)
                    if pi == min(1, npairs - 1) and si + 1 < len(SUPER):
                        emit_wg(si + 1)

    nc.compile()
    return nc


def _host_prep(guidance, depth, conv_w, conv_b, dense_w, dense_b):
    B, H, W, _ = guidance.shape
    nh, nw = H // P, W // P
    NB = B * nh * nw

    def to_samples(x):
        # (B,H,W,F) -> (NB, P, P, F), sample order = flat (b, i, j)
        return (x.reshape(B, nh, P, nw, P, F)
                 .transpose(0, 1, 3, 2, 4, 5)
                 .reshape(NB, P, P, F))

    gs = to_samples(np.ascontiguousarray(guidance))
    ds = to_samples(np.ascontiguousarray(depth))

    in_maps = []
    for c in range(NCORES):
        gsl = gs[c * 512:(c + 1) * 512]
        dsl = ds[c * 512:(c + 1) * 512]
        gpad = np.zeros((SPC, P, P, F), np.float32)
        gpad[:512] = gsl
        dpad = np.zeros((SPC, PADW, PADW, F), np.float32)
        dpad[:512, 1:P + 1, 1:P + 1] = dsl
        # (SPC, y, x, ch) -> [NGROUP, 126, pix]  with q = n_local*9 + ch
        gq = (gpad.reshape(NGROUP, NL, P, P, F)
                  .transpose(1, 4, 0, 2, 3)
                  .reshape(Q, NGROUP, PIX))
        dq = (dpad.reshape(NGROUP, NL, PADW, PADW, F)
                  .transpose(1, 4, 0, 2, 3)
                  .reshape(Q, NGROUP, PPIX))
        in_maps.append({"gin": np.ascontiguousarray(gq).astype(BF16),
                        "din": np.ascontiguousarray(dq).astype(BF16)})

    eye = np.eye(NL, dtype=np.float32)
    lhsA = np.zeros((KS * KS, Q, Q), np.float32)
    for t in range(KS * KS):
        ky, kx = divmod(t, KS)
        lhsA[t] = np.kron(eye, conv_w[ky, kx])
    lhsA = np.ascontiguousarray(lhsA.transpose(1, 0, 2))      # [Q, 9, Q]
    lhsD = np.zeros((F, Q + 1, Q), np.float32)
    lhsD2 = np.zeros((F, Q + 1, Q), np.float32)
    dws = dense_w.astype(np.float32) / PIX  # gap arrives as a SUM over pixels
    for j in range(F):
        lhsD[j, :Q] = np.kron(eye, dws[:, j * F:(j + 1) * F])
        lhsD[j, Q] = np.tile(dense_b[j * F:(j + 1) * F], NL)
        lhsD2[j, :Q] = np.kron(eye, dws[:, j::F])
        lhsD2[j, Q] = np.tile(dense_b[j::F], NL)
    lhsD = np.ascontiguousarray(lhsD.transpose(1, 0, 2))      # [Q+1, 9, Q]
    lhsD2 = np.ascontiguousarray(lhsD2.transpose(1, 0, 2))
    mask = np.kron(eye, np.ones((F, F), np.float32))
    convb = np.tile(conv_b.astype(np.float32), NL)[:, None]

    consts = {"lhsA": lhsA.astype(BF16), "lhsD": lhsD.astype(BF16),
              "lhsD2": lhsD2.astype(BF16),
              "mask": np.ascontiguousarray(mask).astype(BF16),
              "convb": np.ascontiguousarray(convb)}
    for m in in_maps:
        m.update(consts)
    return in_maps


_CACHED_NC = None


def run(inputs, trace=False, **kw):
    """Build (cached), run on 8 cores, return (full_output, BassKernelResults)."""
    global _CACHED_NC
    inputs = {k: np.asarray(v, np.float32) for k, v in inputs.items()}
    in_maps = _host_prep(**inputs)
    if _CACHED_NC is None:
        _CACHED_NC = build_program()
    res = run_bass_kernel_spmd(_CACHED_NC, in_maps, list(range(NCORES)),
                               trace=trace, **kw)
    outs = []
    for c in range(NCORES):
        o = res.results[c]["out"].astype(np.float32).reshape(NL, F, NGROUP, P, P)
        o = o.transpose(2, 0, 3, 4, 1).reshape(SPC, P, P, F)[:512]
        outs.append(o)
    full = np.concatenate(outs, 0)  # (4096, 24, 24, 9) in (b, i, j) order
    B, H, W = 16, 384, 384
    return full.reshape(B, H, W, F), res


def kernel(**inputs):
    out, _ = run(inputs, trace=False)
    return out



# revision 47
# speedup vs baseline: 1.4662x; 1.0539x over previous
"""Trainium2 Bass kernel for the Guided-Conv problem.

Math (per independent sample n, of NB = 4096):
  g_n, d_n : 24x24x9 patches of guidance / depth.
  c_n      = conv2d(g_n, conv_w, stride 8, SAME) + conv_b        -> 3x3x9
  k_n[i]   = c_n[:, :, i] / max(||c_n[:, :, i]||_2, 1)           (per-channel 3x3 filter)
  gap_n    = mean(g_n, (y, x))                                   -> 9
  W2_n     = (gap_n @ dense_w + dense_b).reshape(9, 9)           (i2 -> o2)
  r2_n[o]  = 1 / max(||W2_n[:, o]||_2, 1)
  out_n    = (depthwise(d_n, k_n) @ W2_n) * r2_n                 -> 24x24x9

Device strategy (per core: 512 samples + 6 pad = 37 groups of 14):
  Partition layout q = n_local*9 + ch on 126 partitions; free = pixels.
  - Kernel generation (c_n, W2_n) via block-diagonal matmuls: K = (n,ch),
    lhsT = kron(eye(14), w) built on host, so 14 samples per matmul.
  - Depthwise(3x3) + 1x1 fused: out[(n,o), pix] = sum_{t,i} BD_t[(n,i),(n,o)]
    * d_pad[(n,i), pix+t], 9 tap-matmuls accumulating in PSUM, float32r.
    BD_t = mask (.) (W2row-bcast) (.) k[:, t]  -- one DVE scalar_tensor_tensor.
  - r2 applied for free as the per-partition ACT scale on the PSUM->SBUF copy.
Host does all layout (patch extraction, channel de-interleave, zero-pad) --
this keeps every DMA contiguous in >=2KB runs.
"""

import numpy as np
import ml_dtypes

import concourse.bass as bass
from concourse import bacc
import concourse.mybir as mybir
from concourse.tile import TileContext
from concourse.bass_utils import run_bass_kernel_spmd

BF16 = ml_dtypes.bfloat16

F = 9          # channels
P = 24         # patch size
PADW = 26      # padded patch width (SAME conv, pad 1)
KS = 3         # generated kernel size
NCORES = 8
NL = 14        # samples per group
Q = NL * F     # 126 used partitions
NGROUP = 37    # groups per core (36 full + 1 padded)
SPC = NGROUP * NL  # 518 sample slots per core (512 real)
PIX = P * P        # 576
PPIX = PADW * PADW  # 676
HALF = PIX // 2    # 288, pixels per PSUM chunk (<=512 fp32/bank)
SUPER = [4, 8, 8, 8, 9]   # weight-gen supertile sizes (sum = 37);
                          # small first tile -> main conv starts early

F32 = mybir.dt.float32
BF = mybir.dt.bfloat16


def build_program():
    nc = bacc.Bacc("TRN2", target_bir_lowering=False, debug=False,
                   num_devices=NCORES)

    gin = nc.dram_tensor("gin", [Q, NGROUP, PIX], BF, kind="ExternalInput").ap()
    din = nc.dram_tensor("din", [Q, NGROUP, PPIX], BF, kind="ExternalInput").ap()
    lhsA = nc.dram_tensor("lhsA", [Q, KS * KS, Q], BF, kind="ExternalInput").ap()
    lhsD = nc.dram_tensor("lhsD", [Q + 1, F, Q], BF, kind="ExternalInput").ap()
    lhsD2 = nc.dram_tensor("lhsD2", [Q + 1, F, Q], BF, kind="ExternalInput").ap()
    maskd = nc.dram_tensor("mask", [Q, Q], BF, kind="ExternalInput").ap()
    convbd = nc.dram_tensor("convb", [Q, 1], F32, kind="ExternalInput").ap()
    outd = nc.dram_tensor("out", [Q, NGROUP, PIX], BF, kind="ExternalOutput").ap()

    with TileContext(nc) as tc:
        with (
            nc.allow_low_precision(reason="bf16 pipeline; tol 2e-2"),
            tc.tile_pool(name="consts", bufs=1) as cpool,
            tc.tile_pool(name="gpool", bufs=2) as gpool,
            tc.tile_pool(name="dpool", bufs=8) as dpool,
            tc.tile_pool(name="opool", bufs=6) as opool,
            tc.tile_pool(name="scrap", bufs=2) as scpool,
            tc.tile_pool(name="small", bufs=1) as spool,
            tc.tile_pool(name="bd", bufs=12) as bdpool,
            tc.tile_pool(name="ps_c", bufs=1, space="PSUM") as pcpool,
            tc.tile_pool(name="ps_d", bufs=1, space="PSUM") as pdpool,
            tc.tile_pool(name="ps_main", bufs=3, space="PSUM") as pmpool,
        ):
            # ---- constants ----
            # Fully-contiguous DRAM sources lower to ONE DMA descriptor,
            # which lands on a single DMA engine (~22 GB/s). Chunk by
            # partition range so the transfers spread across engines.
            def dma_const(tile_ap, dram_ap, nchunks):
                pn = tile_ap.shape[0]
                step = (pn + nchunks - 1) // nchunks
                for i in range(0, pn, step):
                    j = min(pn, i + step)
                    nc.sync.dma_start(out=tile_ap[i:j], in_=dram_ap[i:j])

            lhsA_sb = cpool.tile([Q, KS * KS, Q], BF, tag="lhsA")
            lhsD_sb = cpool.tile([Q + 1, F, Q], BF, tag="lhsD")
            lhsD2_sb = cpool.tile([Q + 1, F, Q], BF, tag="lhsD2")
            mask_sb = cpool.tile([Q, NL, F], BF, tag="mask")
            convb_sb = cpool.tile([Q, 1], F32, tag="convb")

            # ---- persistent per-core small tensors ----
            craw = spool.tile([Q, NGROUP, F], F32, tag="craw")     # c + conv_b
            knorm = spool.tile([Q, NGROUP, F], BF, tag="knorm")    # normalized taps
            w2 = spool.tile([Q, NGROUP, F], BF, tag="w2")          # raw W2 (D2 layout)
            r2 = spool.tile([Q, NGROUP], F32, tag="r2")            # 1/max(n2,1)
            r1 = spool.tile([Q, NGROUP], F32, tag="r1")            # 1/max(n1,1)
            sq = spool.tile([Q, NGROUP, F], F32, tag="sq")         # scratch squares
            s1 = spool.tile([Q, NGROUP], F32, tag="s1")            # scratch sums

            starts = []
            _g = 0
            for ng in SUPER:
                starts.append(_g)
                _g += ng

            pre = {}

            def emit_gin(si):
                # prefetch guidance for supertile si + its gap reduction
                ngi = SUPER[si]
                gsli = slice(starts[si], starts[si] + ngi)
                gsb = gpool.tile([Q, ngi * PIX], BF, tag="gsb", bufs=3)
                nc.sync.dma_start(out=gsb,
                                  in_=gin[:, gsli].rearrange("p g f -> p (g f)"))
                # gap: per-group pixel SUM (the 1/576 mean scale is folded
                # into lhsD/lhsD2 on the host). Row 126 must read 1.0 in the
                # K=127 dense matmuls, so memset the whole tile first.
                gap = spool.tile([128, ngi], BF, tag="gap", bufs=3)
                nc.vector.memset(gap, 1.0)
                nc.vector.tensor_reduce(
                    out=gap[0:Q, :],
                    in_=gsb.rearrange("p (g f) -> p g f", g=ngi),
                    axis=mybir.AxisListType.X, op=mybir.AluOpType.add)
                pre[si] = (gsb, gap)

            dpre = {}

            def emit_din(p0, npair):
                dsb = dpool.tile([Q, 2 * PPIX], BF, tag="dsb")
                nc.sync.dma_start(
                    out=dsb[:, :npair * PPIX],
                    in_=din[:, p0:p0 + npair].rearrange("p g f -> p (g f)"))
                dpre[p0] = dsb

            # startup order: guidance + first depth pair first (they gate
            # compute), then the constant chunks, all interleaving on the
            # DMA engines.
            emit_gin(0)
            emit_din(0, 2)
            dma_const(lhsA_sb, lhsA, 4)
            dma_const(mask_sb, maskd.rearrange("p (a b) -> p a b", b=F), 2)
            emit_gin(1)
            dma_const(lhsD_sb, lhsD, 4)
            dma_const(lhsD2_sb, lhsD2, 4)
            nc.gpsimd.dma_start(out=convb_sb, in_=convbd)

            def emit_wg(si):
                # weight generation (KGL conv + dense + norms) for supertile si
                ng = SUPER[si]
                g0 = starts[si]
                gsl = slice(g0, g0 + ng)
                gsb, gap = pre.pop(si)

                # step A: strided conv -> c, 9 accumulated BD matmuls
                psc = pcpool.tile([Q, ng, F], F32, tag="psc")
                gwin = gsb.rearrange(
                    "p (g oy yr ox xr) -> p g oy ox yr xr",
                    g=ng, oy=KS, yr=8, ox=KS, xr=8)
                for t in range(KS * KS):
                    ky, kx = divmod(t, KS)
                    nc.tensor.matmul(
                        psc,
                        lhsT=lhsA_sb[:, t, :],
                        rhs=gwin[:, :, :, :, ky, kx],
                        start=(t == 0), stop=(t == KS * KS - 1),
                        skip_group_check=True)

                # craw = psc + conv_b (per-partition bias)
                nc.scalar.activation(
                    out=craw[:, gsl, :], in_=psc,
                    func=mybir.ActivationFunctionType.Identity,
                    bias=convb_sb, scale=1.0)

                # dense layer, both layouts (D for the norm, D2 for the values)
                psDall = pdpool.tile([Q, 2, F, ng], F32, tag="psDall")
                psD = psDall[:, 0]
                psD2 = psDall[:, 1]
                for j in range(F):
                    nc.tensor.matmul(psD[:, j, :], lhsT=lhsD_sb[:, j, :],
                                     rhs=gap[0:Q + 1, :],
                                     start=True, stop=True,
                                     skip_group_check=True)
                for j in range(F):
                    nc.tensor.matmul(psD2[:, j, :], lhsT=lhsD2_sb[:, j, :],
                                     rhs=gap[0:Q + 1, :],
                                     start=True, stop=True,
                                     skip_group_check=True)

                # r2 = 1/max(||W2[:,o]||, 1):  sum_i2 D^2 per (n,o2)
                nc.scalar.square(out=sq[:, gsl, :],
                                 in_=psD.rearrange("p i g -> p g i"))
                nc.vector.tensor_reduce(
                    out=s1[:, gsl], in_=sq[:, gsl, :],
                    axis=mybir.AxisListType.X, op=mybir.AluOpType.add)
                nc.scalar.sqrt(out=s1[:, gsl], in_=s1[:, gsl])
                nc.vector.tensor_scalar_max(r2[:, gsl], s1[:, gsl], 1.0)
                nc.vector.reciprocal(r2[:, gsl], r2[:, gsl])

                # W2 raw values, group-major
                nc.scalar.copy(out=w2[:, gsl, :],
                               in_=psD2.rearrange("p o g -> p g o"))

                # r1 = 1/max(||c||, 1) per (n, ch); knorm = craw * r1
                nc.scalar.square(out=sq[:, gsl, :], in_=craw[:, gsl, :])
                nc.vector.tensor_reduce(
                    out=s1[:, gsl], in_=sq[:, gsl, :],
                    axis=mybir.AxisListType.X, op=mybir.AluOpType.add)
                nc.scalar.sqrt(out=s1[:, gsl], in_=s1[:, gsl])
                nc.vector.tensor_scalar_max(r1[:, gsl], s1[:, gsl], 1.0)
                nc.vector.reciprocal(r1[:, gsl], r1[:, gsl])
                nc.vector.tensor_mul(
                    out=knorm[:, gsl, :], in0=craw[:, gsl, :],
                    in1=r1[:, gsl].unsqueeze(2).broadcast_to([Q, ng, F]))

            emit_wg(0)
            for si, ng in enumerate(SUPER):
                g0 = starts[si]

                # ---------- main conv, groups in pairs (fewer DMA setups) ----
                npairs = (ng + 1) // 2
                for pi, p0 in enumerate(range(g0, g0 + ng, 2)):
                    npair = min(2, g0 + ng - p0)
                    dsb = dpre.pop(p0, None)
                    if dsb is None:
                        emit_din(p0, npair)
                        dsb = dpre.pop(p0)
                    osb = opool.tile([Q, 2 * PIX], BF, tag="osb")

                    # W2 block-diagonal base for the pair: bdw = mask (.) w2
                    # (one DVE tensor_tensor, 2x mode), then per-tap scaling
                    # bd_t = bdw * k_t via tensor_scalar (4x) spread across
                    # DVE / ACT / GpSimd to balance engine load.
                    bdw = bdpool.tile([Q, 2, NL, F], BF, tag="bdw")
                    nc.vector.tensor_tensor(
                        out=bdw[:, :npair],
                        in0=mask_sb.unsqueeze(1).broadcast_to([Q, npair, NL, F]),
                        in1=w2[:, p0:p0 + npair, :].unsqueeze(2)
                            .broadcast_to([Q, npair, NL, F]),
                        op=mybir.AluOpType.mult)

                    for gl in range(npair):
                        g = p0 + gl
                        drows = dsb[:, gl * PPIX:(gl + 1) * PPIX].rearrange(
                            "p (r c) -> p r c", c=PADW)
                        # bd_t = bdw * k_t, batched: taps 0-5 in one DVE
                        # tensor_tensor, taps 6-8 on GpSimd.
                        bda = bdpool.tile([Q, 6, NL, F], BF, tag="bda")
                        nc.vector.tensor_tensor(
                            out=bda,
                            in0=bdw[:, gl].unsqueeze(1)
                                .broadcast_to([Q, 6, NL, F]),
                            in1=knorm[:, g, 0:6].unsqueeze(2).unsqueeze(3)
                                .broadcast_to([Q, 6, NL, F]),
                            op=mybir.AluOpType.mult)
                        bdb = bdpool.tile([Q, 3, NL, F], BF, tag="bdb")
                        nc.gpsimd.tensor_tensor(
                            out=bdb,
                            in0=bdw[:, gl].unsqueeze(1)
                                .broadcast_to([Q, 3, NL, F]),
                            in1=knorm[:, g, 6:9].unsqueeze(2).unsqueeze(3)
                                .broadcast_to([Q, 3, NL, F]),
                            op=mybir.AluOpType.mult)

                        pm = pmpool.tile([Q, 2, 512], F32, tag="pm")
                        for t in range(KS * KS):
                            ky, kx = divmod(t, KS)
                            src = bda[:, t] if t < 6 else bdb[:, t - 6]
                            lhsT = src.rearrange("p a b -> p (a b)")
                            for h in range(2):
                                rhs = drows[:, h * 12 + ky:h * 12 + ky + 12,
                                            kx:kx + P]
                                nc.tensor.matmul(
                                    pm[:, h, 0:HALF], lhsT=lhsT, rhs=rhs,
                                    start=(t == 0), stop=(t == KS * KS - 1),
                                    skip_group_check=True)

                        nc.scalar.activation(
                            out=osb[:, gl * PIX:(gl + 1) * PIX],
                            in_=pm[:, :, 0:HALF],
                            func=mybir.ActivationFunctionType.Copy,
                            bias=0.0, scale=r2[:, g:g + 1])
                    nc.gpsimd.dma_start(
                        out=outd[:, p0:p0 + npair].rearrange("p g f -> p (g f)"),
                        in_=osb[:, :npair * PIX])
                    if pi == 0 and si + 2 < len(SUPER):
                        emit_gin(si + 2)
                    if pi == min(1, npairs - 1) and si + 1 < len(SUPER):
                        emit_wg(si + 1)

    nc.compile()
    return nc


def _host_prep(guidance, depth, conv_w, conv_b, dense_w, dense_b):
    B, H, W, _ = guidance.shape
    nh, nw = H // P, W // P
    NB = B * nh * nw

    def to_samples(x):
        # (B,H,W,F) -> (NB, P, P, F), sample order = flat (b, i, j)
        return (x.reshape(B, nh, P, nw, P, F)
                 .transpose(0, 1, 3, 2, 4, 5)
                 .reshape(NB, P, P, F))

    gs = to_samples(np.ascontiguousarray(guidance))
    ds = to_samples(np.ascontiguousarray(depth))

    in_maps = []
    for c in range(NCORES):
        gsl = gs[c * 512:(c + 1) * 512]
        dsl = ds[c * 512:(c + 1) * 512]
        gpad = np.zeros((SPC, P, P, F), np.float32)
        gpad[:512] = gsl
        dpad = np.zeros((SPC, PADW, PADW, F), np.float32)
        dpad[:512, 1:P + 1, 1:P + 1] = dsl
        # (SPC, y, x, ch) -> [NGROUP, 126, pix]  with q = n_local*9 + ch
        gq = (gpad.reshape(NGROUP, NL, P, P, F)
                  .transpose(1, 4, 0, 2, 3)
                  .reshape(Q, NGROUP, PIX))
        dq = (dpad.reshape(NGROUP, NL, PADW, PADW, F)
                  .transpose(1, 4, 0, 2, 3)
                  .reshape(Q, NGROUP, PPIX))
        in_maps.append({"gin": np.ascontiguousarray(gq).astype(BF16),
                        "din": np.ascontiguousarray(dq).astype(BF16)})

    eye = np.eye(NL, dtype=np.float32)
    lhsA = np.zeros((KS * KS, Q, Q), np.float32)
    for t in range(KS * KS):
        ky, kx = divmod(t, KS)
        lhsA[t] = np.kron(eye, conv_w[ky, kx])
    lhsA = np.ascontiguousarray(lhsA.transpose(1, 0, 2))      # [Q, 9, Q]
    lhsD = np.zeros((F, Q + 1, Q), np.float32)
    lhsD2 = np.zeros((F, Q + 1, Q), np.float32)
    dws = dense_w.astype(np.float32) / PIX  # gap arrives as a SUM over pixels
    for j in range(F):
        lhsD[j, :Q] = np.kron(eye, dws[:, j * F:(j + 1) * F])
        lhsD[j, Q] = np.tile(dense_b[j * F:(j + 1) * F], NL)
        lhsD2[j, :Q] = np.kron(eye, dws[:, j::F])
        lhsD2[j, Q] = np.tile(dense_b[j::F], NL)
    lhsD = np.ascontiguousarray(lhsD.transpose(1, 0, 2))      # [Q+1, 9, Q]
    lhsD2 = np.ascontiguousarray(lhsD2.transpose(1, 0, 2))
    mask = np.kron(eye, np.ones((F, F), np.float32))
    convb = np.tile(conv_b.astype(np.float32), NL)[:, None]

    consts = {"lhsA": lhsA.astype(BF16), "lhsD": lhsD.astype(BF16),
              "lhsD2": lhsD2.astype(BF16),
              "mask": np.ascontiguousarray(mask).astype(BF16),
              "convb": np.ascontiguousarray(convb)}
    for m in in_maps:
        m.update(consts)
    return in_maps


_CACHED_NC = None


def run(inputs, trace=False, **kw):
    """Build (cached), run on 8 cores, return (full_output, BassKernelResults)."""
    global _CACHED_NC
    inputs = {k: np.asarray(v, np.float32) for k, v in inputs.items()}
    in_maps = _host_prep(**inputs)
    if _CACHED_NC is None:
        _CACHED_NC = build_program()
    res = run_bass_kernel_spmd(_CACHED_NC, in_maps, list(range(NCORES)),
                               trace=trace, **kw)
    outs = []
    for c in range(NCORES):
        o = res.results[c]["out"].astype(np.float32).reshape(NL, F, NGROUP, P, P)
        o = o.transpose(2, 0, 3, 4, 1).reshape(SPC, P, P, F)[:512]
        outs.append(o)
    full = np.concatenate(outs, 0)  # (4096, 24, 24, 9) in (b, i, j) order
    B, H, W = 16, 384, 384
    return full.reshape(B, H, W, F), res


def kernel(**inputs):
    out, _ = run(inputs, trace=False)
    return out

